# revision 4
# baseline (speedup 1.0000x reference)
"""Bass/Trainium2 kernel for nn_MD_LSTM (2-layer LSTM encoder + autoregressive decoder).

v2: tensor-parallel over the 4H gate dim across 8 cores (each core owns 128
hidden units/layer = 512 gate rows), bf16 weights + bf16 hidden-state
exchange, biases folded into matmuls (rank-2 indicator trick) or ACT bias,
per-gate activations overlapped with the matmul stream (o-gate last), and the
per-step hidden-state AllGather done either via remote_dma_broadcast
SBUF->SBUF (MODE="rdma", ~2us) or via collective_compute (MODE="cc", ~7us).

Layout notes:
  - states kept transposed [H, B]; B=256 is the matmul moving dim.
  - psum banks: cell A=[P,2B] (i,f gates), B=[P,B] (g), C=[P,B] (o);
    2 cells = 6 banks, + enc proj bank + dec proj bank = 8.
  - gates i,f biases enter via one K=2 matmul (bias^T stationary, indicator
    rhs); g,o biases via the ACT bias port. tanh used directly (same ACT
    table set as sigmoid - no table thrash).
"""
import sys
sys.path.insert(0, "/opt/trn_rl_repo")
import numpy as np
import ml_dtypes

P = 128
B, S, F, H = 256, 100, 256, 1024
NCORES = 8
NG = 4
NKH = H // P          # 8
NKF = F // P          # 2
NFT = F // P          # 2
FS = F // NCORES      # 32 enc-proj rows per core

MODE = "cc"           # "rdma" | "cc"
_CACHE = {}
TRACE = False
TRACE_DIR = None
LAST = None


def _build(n_enc, n_dec):
    import concourse.bass as bass
    import concourse.tile as tile
    from concourse.tile import add_dep_helper
    from concourse import bacc, mybir

    dt = mybir.dt
    AF = mybir.ActivationFunctionType
    f32, bf16 = dt.float32, dt.bfloat16

    nc = bacc.Bacc("TRN2", target_bir_lowering=False, debug=False,
                   enable_asserts=True, num_devices=NCORES,
                   monotonic_sem_count=3)

    # ---- I/O ----
    d_xt = nc.dram_tensor("xt", [n_enc, NKF, P, B], bf16, kind="ExternalInput")
    d_xsl = nc.dram_tensor("xsl", [n_enc, FS, B], f32, kind="ExternalInput")
    d_xlast = nc.dram_tensor("xlast", [P, NFT, B], f32, kind="ExternalInput")
    d_wih0 = nc.dram_tensor("wih0t", [P, NKF, NG, P], bf16, kind="ExternalInput")
    d_whh0 = nc.dram_tensor("whh0t", [P, NKH, NG, P], bf16, kind="ExternalInput")
    d_wih1 = nc.dram_tensor("wih1t", [P, NKH, NG, P], bf16, kind="ExternalInput")
    d_whh1 = nc.dram_tensor("whh1t", [P, NKH, NG, P], bf16, kind="ExternalInput")
    d_wout = nc.dram_tensor("woutsh", [P, NKH, FS], bf16, kind="ExternalInput")
    d_woutL = nc.dram_tensor("woutL", [P, NKH, NFT, P], bf16, kind="ExternalInput")
    d_woutd = nc.dram_tensor("woutd", [P, NKH, NFT, P], bf16, kind="ExternalInput")
    d_bif0 = nc.dram_tensor("bif0", [2, P], bf16, kind="ExternalInput")
    d_bif1 = nc.dram_tensor("bif1", [2, P], bf16, kind="ExternalInput")
    d_bg0 = nc.dram_tensor("bg0", [P, 1], f32, kind="ExternalInput")
    d_bo0 = nc.dram_tensor("bo0", [P, 1], f32, kind="ExternalInput")
    d_bg1 = nc.dram_tensor("bg1", [P, 1], f32, kind="ExternalInput")
    d_bo1 = nc.dram_tensor("bo1", [P, 1], f32, kind="ExternalInput")
    d_ind = nc.dram_tensor("ind", [2, 2 * B], bf16, kind="ExternalInput")
    d_cdecT = nc.dram_tensor("cdecT", [2, P], bf16, kind="ExternalInput")
    d_outbs = nc.dram_tensor("outbs", [FS, 1], f32, kind="ExternalInput")
    d_outbF = nc.dram_tensor("outbF", [P, NFT], f32, kind="ExternalInput")

    d_enc = nc.dram_tensor("enc_out", [n_enc, FS, B], f32, kind="ExternalOutput")
    d_enclast = nc.dram_tensor("enc_last", [P, NFT, B], f32, kind="ExternalOutput")
    d_dec = nc.dram_tensor("dec_out", [n_dec, NFT, P, B], f32, kind="ExternalOutput")

    # collective staging (cc mode), ping-pong x2 per exchange class
    if MODE == "cc":
        d_cci = [[nc.dram_tensor(f"cc{w}i{i}", [P, B], bf16) for i in range(2)]
                 for w in range(2)]
        d_cco = [[nc.dram_tensor(f"cc{w}o{i}", [NCORES * P, B], bf16)
                  for i in range(2)] for w in range(2)]
    rg = [list(range(NCORES))]
    RD = [(0, k) for k in range(NCORES)]

    with tile.TileContext(nc) as tc:
        with (
            tc.tile_pool(name="wp", bufs=1) as wp,        # persistent weights/state
            tc.tile_pool(name="xt", bufs=3) as xtp,       # per-step x tiles
            tc.tile_pool(name="xs", bufs=3) as xsp,       # per-step x slice (enc out)
            tc.tile_pool(name="pw", bufs=2) as pwp,       # pointwise temps
            tc.tile_pool(name="ac", bufs=2) as acp,       # activation outputs
            tc.tile_pool(name="hs", bufs=4) as hsp,       # h send tiles
            tc.tile_pool(name="eo", bufs=3) as eop,       # enc out staging
            tc.tile_pool(name="nt", bufs=2) as ntp,       # dec new staging
            tc.tile_pool(name="pg", bufs=1, space="PSUM") as pg,
        ):
            # ---- persistent tiles ----
            w_ih0 = wp.tile([P, NKF, NG, P], bf16)
            w_hh0 = wp.tile([P, NKH, NG, P], bf16)
            w_ih1 = wp.tile([P, NKH, NG, P], bf16)
            w_hh1 = wp.tile([P, NKH, NG, P], bf16)
            w_out = wp.tile([P, NKH, FS], bf16)
            w_outL = wp.tile([P, NKH, NFT, P], bf16)
            w_outd = wp.tile([P, NKH, NFT, P], bf16)
            nc.sync.dma_start(w_ih0[:], d_wih0[:])
            nc.sync.dma_start(w_hh0[:], d_whh0[:])
            nc.sync.dma_start(w_ih1[:], d_wih1[:])
            nc.sync.dma_start(w_hh1[:], d_whh1[:])
            nc.sync.dma_start(w_out[:], d_wout[:])
            nc.sync.dma_start(w_outL[:], d_woutL[:])
            nc.sync.dma_start(w_outd[:], d_woutd[:])
            b_if0 = wp.tile([2, P], bf16)
            b_if1 = wp.tile([2, P], bf16)
            cdecT = wp.tile([2, P], bf16)
            ind = wp.tile([2, 2 * B], bf16)
            nc.sync.dma_start(b_if0[:], d_bif0[:])
            nc.sync.dma_start(b_if1[:], d_bif1[:])
            nc.sync.dma_start(cdecT[:], d_cdecT[:])
            nc.sync.dma_start(ind[:], d_ind[:])
            b_g0 = wp.tile([P, 1], f32)
            b_o0 = wp.tile([P, 1], f32)
            b_g1 = wp.tile([P, 1], f32)
            b_o1 = wp.tile([P, 1], f32)
            outbs = wp.tile([FS, 1], f32)
            outbF = wp.tile([P, NFT], f32)
            nc.sync.dma_start(b_g0[:], d_bg0[:])
            nc.sync.dma_start(b_o0[:], d_bo0[:])
            nc.sync.dma_start(b_g1[:], d_bg1[:])
            nc.sync.dma_start(b_o1[:], d_bo1[:])
            nc.sync.dma_start(outbs[:], d_outbs[:])
            nc.sync.dma_start(outbF[:], d_outbF[:])

            # gathered h state, ping-pong x2 per layer; [P, slot*B] bf16
            h0g = [wp.tile([P, NCORES * B], bf16, name=f"h0g{i}") for i in range(2)]
            h1g = [wp.tile([P, NCORES * B], bf16, name=f"h1g{i}") for i in range(2)]
            for t_ in h0g + h1g:
                nc.gpsimd.memset(t_[:], 0.0)
            c0 = wp.tile([P, B], f32)
            c1 = wp.tile([P, B], f32)
            nc.gpsimd.memset(c0[:], 0.0)
            nc.gpsimd.memset(c1[:], 0.0)
            lastF = wp.tile([P, NFT, B], f32)
            lastB = wp.tile([P, NFT, B], bf16)

            if MODE == "rdma":
                mono_r0 = nc.monotonic_semaphore(0)
                mono_r1 = nc.monotonic_semaphore(1)
                mono_s = nc.monotonic_semaphore(2)
                slot_off = nc.gpsimd.partition_id() * B

            ACT_SIG, ACT_TANH = AF.Sigmoid, AF.Tanh

            def exchange(hsend, gbuf, which, par):
                """AllGather hsend [P,B] bf16 into gbuf [P, NCORES*B]."""
                if MODE == "rdma":
                    mono = mono_r0 if which == 0 else mono_r1
                    with tc.tile_critical():
                        nc.gpsimd.remote_dma_broadcast(
                            out_ap=gbuf[:, bass.ds(slot_off, B)],
                            in_ap=hsend[:],
                            remote_sem=mono.sem(),
                            local_sem=mono_s.sem(),
                            rdests=RD,
                        )
                        tc.wait_critical_data_deps()
                        nc.gpsimd.trigger_dma(count=None)
                        mono.wait_inc(16)
                else:
                    cci = d_cci[which][par]
                    cco = d_cco[which][par]
                    din = nc.sync.dma_start(cci[:], hsend[:])
                    nc.gpsimd.collective_compute(
                        "AllGather", mybir.AluOpType.bypass, replica_groups=rg,
                        ins=[cci.ap().opt()], outs=[cco.ap().opt()])
                    gv = gbuf[:].rearrange("p (o b) -> p o b", o=NCORES)
                    cv = cco.ap().rearrange("(o p) b -> p o b", p=P)
                    H8 = NCORES // 2
                    nc.sync.dma_start(gv[:, 0:H8], cv[:, 0:H8])
                    nc.scalar.dma_start(gv[:, H8:], cv[:, H8:])
                    return din

            def cell0_pre(psA, psB, psC, h0prev, gate_on=None):
                """bias + hh0 matmuls (only needs h0g(t-1)). If gate_on is a
                DMA instruction, the block is held until it completes so the
                PE runs it inside the exchange-flight window."""
                first = nc.tensor.matmul(psA[:], b_if0[:], ind[:], start=True,
                                         stop=False)
                if gate_on is not None:
                    add_dep_helper(first.ins, gate_on.ins,
                                   reason="gate hh0 prefill into AG window")
                for g, ps, col in ((0, psA, 0), (1, psA, B)):
                    for kh in range(NKH):
                        nc.tensor.matmul(ps[:, col:col + B], w_hh0[:, kh, g, :],
                                         h0prev[:, kh * B:(kh + 1) * B],
                                         start=False, stop=False)
                for kh in range(NKH):
                    nc.tensor.matmul(psB[:], w_hh0[:, kh, 2, :],
                                     h0prev[:, kh * B:(kh + 1) * B],
                                     start=(kh == 0), stop=False)
                for kh in range(NKH):
                    nc.tensor.matmul(psC[:], w_hh0[:, kh, 3, :],
                                     h0prev[:, kh * B:(kh + 1) * B],
                                     start=(kh == 0), stop=False)

            def cell0_post(psA, psB, psC, rhs2):
                """input-path matmuls Wih0 @ rhs2 (x_t or lastB); sets stops.

                rhs2: list of NKF APs [P, B]."""
                for g, ps, col in ((0, psA, 0), (1, psA, B)):
                    for k2 in range(NKF):
                        nc.tensor.matmul(ps[:, col:col + B], w_ih0[:, k2, g, :],
                                         rhs2[k2],
                                         start=False,
                                         stop=(g == 1 and k2 == NKF - 1))
                for k2 in range(NKF):
                    nc.tensor.matmul(psB[:], w_ih0[:, k2, 2, :], rhs2[k2],
                                     start=False, stop=(k2 == NKF - 1))
                for k2 in range(NKF):
                    nc.tensor.matmul(psC[:], w_ih0[:, k2, 3, :], rhs2[k2],
                                     start=False, stop=(k2 == NKF - 1))

            def cell_pointwise(psA, psB, psC, bg, bo, cstate, hout_dt):
                """LSTM pointwise: returns h tile (dtype hout_dt [P, B])."""
                aif = acp.tile([P, 2 * B], f32, name="aif")
                nc.scalar.activation(aif[:], psA[:], ACT_SIG)
                tg = pwp.tile([P, B], f32, name="tg")
                nc.scalar.activation(tg[:], psB[:], ACT_TANH, bias=bg[:])
                so = pwp.tile([P, B], f32, name="so")
                nc.scalar.activation(so[:], psC[:], ACT_SIG, bias=bo[:])
                t1 = pwp.tile([P, B], f32, name="t1")
                t2 = pwp.tile([P, B], f32, name="t2")
                nc.vector.tensor_mul(t1[:], aif[:, B:2 * B], cstate[:])
                nc.vector.tensor_mul(t2[:], aif[:, 0:B], tg[:])
                nc.vector.tensor_add(cstate[:], t1[:], t2[:])
                tc_ = pwp.tile([P, B], f32, name="tc")
                nc.scalar.activation(tc_[:], cstate[:], ACT_TANH)
                h = hsp.tile([P, B], hout_dt, name="hsend")
                nc.vector.tensor_mul(h[:], so[:], tc_[:])
                return h

            def l1_mms_pre(psA, psB, psC, h1prev, gate_on=None):
                """hh1 part of layer 1 (issued early: only needs h1g(t-1))."""
                first = nc.tensor.matmul(psA[:], b_if1[:], ind[:], start=True,
                                         stop=False)
                if gate_on is not None:
                    add_dep_helper(first.ins, gate_on.ins,
                                   reason="gate hh1 prefill into AG window")
                for g, ps, col in ((0, psA, 0), (1, psA, B)):
                    for kh in range(NKH):
                        nc.tensor.matmul(ps[:, col:col + B], w_hh1[:, kh, g, :],
                                         h1prev[:, kh * B:(kh + 1) * B],
                                         start=False, stop=False)
                for kh in range(NKH):
                    nc.tensor.matmul(psB[:], w_hh1[:, kh, 2, :],
                                     h1prev[:, kh * B:(kh + 1) * B],
                                     start=(kh == 0), stop=False)
                for kh in range(NKH):
                    nc.tensor.matmul(psC[:], w_hh1[:, kh, 3, :],
                                     h1prev[:, kh * B:(kh + 1) * B],
                                     start=(kh == 0), stop=False)

            def l1_mms_post(psA, psB, psC, h0cur):
                """ih1 part of layer 1 (needs h0g(t))."""
                for g, ps, col in ((0, psA, 0), (1, psA, B)):
                    for kh in range(NKH):
                        nc.tensor.matmul(ps[:, col:col + B], w_ih1[:, kh, g, :],
                                         h0cur[:, kh * B:(kh + 1) * B],
                                         start=False,
                                         stop=(g == 1 and kh == NKH - 1))
                for kh in range(NKH):
                    nc.tensor.matmul(psB[:], w_ih1[:, kh, 2, :],
                                     h0cur[:, kh * B:(kh + 1) * B],
                                     start=False, stop=(kh == NKH - 1))
                for kh in range(NKH):
                    nc.tensor.matmul(psC[:], w_ih1[:, kh, 3, :],
                                     h0cur[:, kh * B:(kh + 1) * B],
                                     start=False, stop=(kh == NKH - 1))

            def emit_enc_proj(t, h1cur):
                """Sharded dx(t) proj + enc output rows [FS] for step t."""
                pj = pg.tile([FS, B], f32, name="pjE")
                for kh in range(NKH):
                    nc.tensor.matmul(pj[:], w_out[:, kh, :],
                                     h1cur[:, kh * B:(kh + 1) * B],
                                     start=(kh == 0), stop=(kh == NKH - 1))
                xs = xsp.tile([FS, B], f32, name="xs")
                nc.sync.dma_start(xs[:], d_xsl[t])
                eo = eop.tile([FS, B], f32, name="eo")
                nc.vector.tensor_add(eo[:], pj[:], xs[:])
                nc.vector.tensor_scalar_add(eo[:], eo[:], outbs[:])
                nc.sync.dma_start(d_enc[t], eo[:])

            def emit_dec_proj(j, h1cur):
                """new(j) = lastF + woutd@h1(j) + cdec; updates lastF/lastB,
                writes dec_out[j]."""
                pj = pg.tile([P, NFT * B], f32, name="pjD")
                nc.tensor.matmul(pj[:], cdecT[:], ind[:], start=True, stop=False)
                for ft in range(NFT):
                    for kh in range(NKH):
                        nc.tensor.matmul(pj[:, ft * B:(ft + 1) * B],
                                         w_outd[:, kh, ft, :],
                                         h1cur[:, kh * B:(kh + 1) * B],
                                         start=False,
                                         stop=(ft == NFT - 1 and kh == NKH - 1))
                newt = ntp.tile([P, NFT, B], f32, name="newt")
                for ft in range(NFT):
                    nc.vector.tensor_add(newt[:, ft, :],
                                         pj[:, ft * B:(ft + 1) * B],
                                         lastF[:, ft, :])
                nc.vector.tensor_copy(lastF[:], newt[:])
                nc.vector.tensor_copy(lastB[:], newt[:])
                nc.sync.dma_start(d_dec[j].rearrange("f p b -> p f b"), newt[:])

            # ================= encoder =================
            din1 = None
            for t in range(n_enc):
                pu = t % 2          # gbuf parity being written this step
                pv = (t - 1) % 2    # gbuf parity read (prev step state)
                psA = pg.tile([P, 2 * B], f32, name="c0A")
                psB = pg.tile([P, B], f32, name="c0B")
                psC = pg.tile([P, B], f32, name="c0C")
                xt = xtp.tile([P, NKF, B], bf16, name="xt")
                nc.sync.dma_start(xt[:], d_xt[t].rearrange("a p b -> p a b"))
                cell0_pre(psA, psB, psC, h0g[pv], gate_on=din1)
                cell0_post(psA, psB, psC, [xt[:, 0, :], xt[:, 1, :]])
                h0s = cell_pointwise(psA, psB, psC, b_g0, b_o0, c0, bf16)
                din0 = exchange(h0s, h0g[pu], 0, pu)

                qA = pg.tile([P, 2 * B], f32, name="c1A")
                qB = pg.tile([P, B], f32, name="c1B")
                qC = pg.tile([P, B], f32, name="c1C")
                l1_mms_pre(qA, qB, qC, h1g[pv], gate_on=din0)
                if t > 0:
                    emit_enc_proj(t - 1, h1g[pv])
                l1_mms_post(qA, qB, qC, h0g[pu])
                h1s = cell_pointwise(qA, qB, qC, b_g1, b_o1, c1, bf16)
                din1 = exchange(h1s, h1g[pu], 1, pu)

            # last enc step: full (replicated) projection -> lastF/lastB + outputs
            pvL = (n_enc - 1) % 2
            pjL = pg.tile([P, NFT * B], f32, name="pjD")
            for ft in range(NFT):
                for kh in range(NKH):
                    nc.tensor.matmul(pjL[:, ft * B:(ft + 1) * B],
                                     w_outL[:, kh, ft, :],
                                     h1g[pvL][:, kh * B:(kh + 1) * B],
                                     start=(kh == 0), stop=(kh == NKH - 1))
            xlast = wp.tile([P, NFT, B], f32)
            nc.sync.dma_start(xlast[:], d_xlast[:])
            for ft in range(NFT):
                tmp = pwp.tile([P, B], f32, name="tmpL")
                nc.vector.tensor_scalar_add(tmp[:], pjL[:, ft * B:(ft + 1) * B],
                                            outbF[:, ft:ft + 1])
                nc.vector.tensor_add(lastF[:, ft, :], tmp[:], xlast[:, ft, :])
            nc.vector.tensor_copy(lastB[:], lastF[:])
            nc.sync.dma_start(d_enclast[:], lastF[:])

            # ================= decoder =================
            # step j: proj(j-1) lands between hh0-prefill and ih0 so the PE
            # has work during the h1-exchange flight.
            for j in range(n_dec):
                t = n_enc + j
                pu = t % 2
                pv = (t - 1) % 2
                psA = pg.tile([P, 2 * B], f32, name="c0A")
                psB = pg.tile([P, B], f32, name="c0B")
                psC = pg.tile([P, B], f32, name="c0C")
                cell0_pre(psA, psB, psC, h0g[pv], gate_on=din1)
                if j > 0:
                    emit_dec_proj(j - 1, h1g[pv])
                cell0_post(psA, psB, psC, [lastB[:, 0, :], lastB[:, 1, :]])
                h0s = cell_pointwise(psA, psB, psC, b_g0, b_o0, c0, bf16)
                din0 = exchange(h0s, h0g[pu], 0, pu)

                qA = pg.tile([P, 2 * B], f32, name="c1A")
                qB = pg.tile([P, B], f32, name="c1B")
                qC = pg.tile([P, B], f32, name="c1C")
                l1_mms_pre(qA, qB, qC, h1g[pv], gate_on=din0)
                l1_mms_post(qA, qB, qC, h0g[pu])
                h1s = cell_pointwise(qA, qB, qC, b_g1, b_o1, c1, bf16)
                din1 = exchange(h1s, h1g[pu], 1, pu)

            emit_dec_proj(n_dec - 1, h1g[(n_enc + n_dec - 1) % 2])

    nc.compile()
    return nc


def _prep_inputs(x, W_ih0, W_hh0, b0, W_ih1, W_hh1, b1, out_W, out_b, dy_mu, dy_std,
                 n_enc, core):
    """Host-side transposes/shards for one core."""
    bf16 = ml_dtypes.bfloat16
    k = core

    def gate_slices(W, nk):
        w4 = W.reshape(NG, H, W.shape[1])[:, k * P:(k + 1) * P, :]   # [NG,P(m),K]
        return np.ascontiguousarray(
            w4.reshape(NG, P, nk, P).transpose(3, 2, 0, 1)).astype(bf16)

    out_Wd = out_W * dy_std[:, None]
    # xt: [n_enc, NKF, P, B] bf16 (x transposed, f-major k-tiles)
    xt = np.ascontiguousarray(
        x[:, :n_enc].transpose(2, 1, 0)                               # [F,S,B]
        .reshape(NKF, P, n_enc, B).transpose(2, 0, 1, 3)).astype(bf16)
    # xsl: per-core F rows [32k..32k+32): [n_enc, FS, B]
    xsl = np.ascontiguousarray(
        x[:, :n_enc, k * FS:(k + 1) * FS].transpose(1, 2, 0))
    # xlast: full x at last enc step: [P, NFT, B]  (row index = ft*P + p)
    xlast = np.ascontiguousarray(
        x[:, n_enc - 1, :].T.reshape(NFT, P, B).transpose(1, 0, 2))

    woutsh = np.ascontiguousarray(
        out_W[k * FS:(k + 1) * FS].reshape(FS, NKH, P)
        .transpose(2, 1, 0)).astype(bf16)                             # [P,NKH,FS]
    woutL = np.ascontiguousarray(
        out_W.reshape(NFT, P, NKH, P).transpose(3, 2, 0, 1)).astype(bf16)
    woutd = np.ascontiguousarray(
        out_Wd.reshape(NFT, P, NKH, P).transpose(3, 2, 0, 1)).astype(bf16)

    b0r = b0.reshape(NG, H)[:, k * P:(k + 1) * P]                     # [4,P]
    b1r = b1.reshape(NG, H)[:, k * P:(k + 1) * P]
    cdec = (dy_std * out_b + dy_mu).reshape(NFT, P)                   # [2,P]

    ind = np.zeros((2, 2 * B), np.float32)
    ind[0, :B] = 1.0
    ind[1, B:] = 1.0

    return {
        "xt": xt, "xsl": xsl, "xlast": xlast,
        "wih0t": gate_slices(W_ih0, NKF),
        "whh0t": gate_slices(W_hh0, NKH),
        "wih1t": gate_slices(W_ih1, NKH),
        "whh1t": gate_slices(W_hh1, NKH),
        "woutsh": woutsh, "woutL": woutL, "woutd": woutd,
        "bif0": np.ascontiguousarray(b0r[0:2]).astype(bf16),
        "bif1": np.ascontiguousarray(b1r[0:2]).astype(bf16),
        "bg0": np.ascontiguousarray(b0r[2][:, None]),
        "bo0": np.ascontiguousarray(b0r[3][:, None]),
        "bg1": np.ascontiguousarray(b1r[2][:, None]),
        "bo1": np.ascontiguousarray(b1r[3][:, None]),
        "ind": ind.astype(bf16),
        "cdecT": np.ascontiguousarray(cdec).astype(bf16),
        "outbs": np.ascontiguousarray(out_b[k * FS:(k + 1) * FS][:, None]),
        "outbF": np.ascontiguousarray(out_b.reshape(NFT, P).T),
    }


def run_device(x, W_ih0, W_hh0, b0, W_ih1, W_hh1, b1, out_W, out_b, dy_mu, dy_std,
               n_enc, n_dec):
    from concourse.bass_utils import run_bass_kernel_spmd
    global LAST
    key = (n_enc, n_dec, MODE)
    if key not in _CACHE:
        _CACHE[key] = _build(n_enc, n_dec)
    nc = _CACHE[key]
    in_maps = [
        _prep_inputs(x, W_ih0, W_hh0, b0, W_ih1, W_hh1, b1, out_W, out_b,
                     dy_mu, dy_std, n_enc, k)
        for k in range(NCORES)
    ]
    res = run_bass_kernel_spmd(nc, in_maps, core_ids=list(range(NCORES)),
                               trace=TRACE, tmpdir=TRACE_DIR)
    if TRACE:
        LAST = res
    return res


def kernel(**inputs):
    x = np.asarray(inputs["x"], np.float32)
    t = int(np.asarray(inputs["t"]))
    args = [np.asarray(inputs[k], np.float32) for k in
            ["W_ih0", "W_hh0", "b0", "W_ih1", "W_hh1", "b1",
             "out_W", "out_b", "dy_mu", "dy_std"]]
    n_enc, n_dec = x.shape[1], t - 1
    res = run_device(x, *args, n_enc, n_dec)
    # assemble encoder output [B, n_enc, F]
    enc = np.empty((B, n_enc, F), np.float32)
    for k in range(NCORES):
        # d_enc[t] = [FS, B] rows k*FS..(k+1)*FS  (steps 0..n_enc-2)
        e = res.results[k]["enc_out"]                       # [n_enc, FS, B]
        enc[:, :n_enc - 1, k * FS:(k + 1) * FS] = e[:n_enc - 1].transpose(2, 0, 1)
    el = res.results[0]["enc_last"]                         # [P, NFT, B]
    enc[:, n_enc - 1, :] = el.transpose(1, 0, 2).reshape(F, B).T
    dec = res.results[0]["dec_out"]                         # [n_dec, NFT, P, B]
    dec_b = np.ascontiguousarray(dec.transpose(3, 0, 1, 2)).reshape(B, n_dec, F)
    return np.concatenate([x[:, :1], enc, dec_b], axis=1)
